# revision 5
# baseline (speedup 1.0000x reference)
"""Trainium2 Bass kernel v6: conv2d(3x3, VALID) + bias -> channel-min -> tanh(tanh).

Problem shapes (fixed):
  x      [32, 64, 128, 128] f32   (N, C_in, H, W)
  weight [128, 64, 3, 3]    f32   (C_out, C_in, kh, kw)
  bias   [128]              f32
  out    [32, 1, 126, 126]  f32

Strategy (v6: fp8 DoubleRow conv; channel-min split between GpSimd and PE)
--------------------------------------------------------------------------
Data-parallel over 8 cores: 4 images per core, weights/bias replicated.

Conv identical to v5: compact fp8 dup row-block layouts P_AB / P_C
(col = r*126 + w), 3 accumulating matmuls per 504-px tile (2x DoubleRow
K=256 with an overlapping [[pitch,128],[126,2],[1,504]] rhs AP + 1 plain
K=128), negated weights; ScalarE evacuates PSUM per 2-tile block with
m = tanh(-(y+b)) (f32 bias).

Channel-min (min = -max of negated, tanh monotone+odd), split to balance
engines (GpSimd partition_all_reduce has a high fixed+per-element cost;
TensorE has headroom):
  px 0..12095      GpSimdE: one partition_all_reduce(max) per image,
                   scatter row 0 via DRAM round-trip, ScalarE tanh(-.),
                   one output DMA.
  px 12096..15875  TensorE: transpose 128-px chunks of m (channels -> free
                   dim), VectorE max-reduce per chunk, ScalarE tanh(-.),
                   one more transpose so pixels are contiguous, DMA out.
"""

import numpy as np

import concourse.bacc as bacc
import concourse.bass as bass
import concourse.bass_isa as bass_isa
import concourse.tile as tile
from concourse import mybir
from concourse.bass_utils import run_bass_kernel_spmd

N_CORES = 8
N_IMGS = 32
IMGS_PER_CORE = N_IMGS // N_CORES
C_IN = 64
C_OUT = 128
H = W = 128
HO = WO = 126
NPIX = HO * WO  # 15876
NBLK = 131  # row blocks in the dup layouts (126..130 only partially used)
PITCH = NBLK * WO  # 16506
PSPLIT = 12096  # px 0..PSPLIT-1 via GpSimd (96 rows), rest via PE transposes
F8 = mybir.dt.float8e4
F16 = mybir.dt.float16
F32 = mybir.dt.float32


def build_kernel(reps=1, timing=False):
    """reps > 1 repeats the whole per-core compute in one NEFF (for HW timing).

    timing=True declares the big image inputs and the result as Internal DRAM
    (zero-initialized on device at NEFF start) so per-call host<->device
    transfer is tiny; the per-rep instruction stream is identical."""
    nc = bacc.Bacc(trn_type="TRN2", target_bir_lowering=False, debug=False)
    io_kind = "Internal" if timing else None
    pab_d = nc.dram_tensor(
        "pab", [IMGS_PER_CORE, 128, PITCH], F8, kind=io_kind or "ExternalInput"
    )
    pc_d = nc.dram_tensor(
        "pcd", [IMGS_PER_CORE, 128, PITCH], F8, kind=io_kind or "ExternalInput"
    )
    wp = nc.dram_tensor("wp", [128, 5, 128], F8, kind="ExternalInput")
    bias = nc.dram_tensor("bias", [128, 1], F32, kind="ExternalInput")
    ident = nc.dram_tensor("ident", [128, 128], F16, kind="ExternalInput")
    out = nc.dram_tensor(
        "out", [IMGS_PER_CORE, NPIX], F16, kind=io_kind or "ExternalOutput"
    )
    sink = (
        nc.dram_tensor("sink", [1, 64], F32, kind="ExternalOutput") if timing else None
    )
    scratch = nc.dram_tensor("scratch", [96, 126], F16, kind="Internal")

    DR = mybir.MatmulPerfMode.DoubleRow

    with tile.TileContext(nc) as tc:
        with (
            tc.tile_pool(name="consts", bufs=1) as consts,
            tc.tile_pool(name="dpool", bufs=2) as dpool,
            tc.tile_pool(name="mpool", bufs=2) as mpool,
            tc.tile_pool(name="rpool", bufs=2) as rpool,
            tc.tile_pool(name="spool", bufs=2) as spool,
            tc.tile_pool(name="fpool", bufs=2) as fpool,
            tc.tile_pool(name="gpool", bufs=2) as gpool,
            tc.tile_pool(name="pcpool", bufs=2, space="PSUM") as pcpool,
            tc.tile_pool(name="tpool", bufs=2, space="PSUM") as tpool,
            tc.tile_pool(name="gtpool", bufs=2, space="PSUM") as gtpool,
        ):
            wpt = consts.tile([128, 5, 128], F8)
            nc.gpsimd.dma_start(out=wpt[:], in_=wp.ap())
            bt = consts.tile([128, 1], F32)
            nc.gpsimd.dma_start(out=bt[:], in_=bias.ap())
            idt = consts.tile([128, 128], F16)
            nc.gpsimd.dma_start(out=idt[:], in_=ident.ap())

            if timing:
                # one-time (outside the rep loop): zero-fill the internal
                # image inputs; produce the tiny external output
                z = dpool.tile([128, PITCH], F8, tag="pab")
                for q in range(2):
                    nc.vector.memset(z[:, q * 8253 : (q + 1) * 8253], 0.0)
                for img in range(IMGS_PER_CORE):
                    nc.sync.dma_start(out=pab_d.ap()[img], in_=z[:])
                    nc.sync.dma_start(out=pc_d.ap()[img], in_=z[:])
                zs = fpool.tile([1, 64], F32, tag="sink")
                nc.vector.memset(zs[:], 0.0)
                nc.sync.dma_start(out=sink.ap(), in_=zs[:])

            for img in [i for _ in range(reps) for i in range(IMGS_PER_CORE)]:
                pab = dpool.tile([128, PITCH], F8, tag="pab")
                nc.sync.dma_start(out=pab[:], in_=pab_d.ap()[img])
                pct = dpool.tile([128, PITCH], F8, tag="pct")
                nc.sync.dma_start(out=pct[:], in_=pc_d.ap()[img])
                pab_t = pab[:, 0:504].tensor
                pct_t = pct[:, 0:504].tensor

                m = mpool.tile([128, 32 * 504], F16, tag="m")
                for blk in range(16):
                    pcb = pcpool.tile([128, 2, 512], F32, tag="pc")
                    for j in range(2):
                        t = blk * 2 + j
                        rhs_a = bass.AP(
                            tensor=pab_t,
                            offset=504 * t,
                            ap=[[PITCH, 128], [WO, 2], [1, 504]],
                        )
                        rhs_b = bass.AP(
                            tensor=pct_t,
                            offset=504 * t,
                            ap=[[PITCH, 128], [WO, 2], [1, 504]],
                        )
                        nc.tensor.matmul(
                            pcb[:, j, 0:504],
                            lhsT=wpt[:, 0:2, :],
                            rhs=rhs_a,
                            start=True,
                            stop=False,
                            perf_mode=DR,
                        )
                        nc.tensor.matmul(
                            pcb[:, j, 0:504],
                            lhsT=wpt[:, 2:4, :],
                            rhs=rhs_b,
                            start=False,
                            stop=False,
                            perf_mode=DR,
                        )
                        nc.tensor.matmul(
                            pcb[:, j, 0:504],
                            lhsT=wpt[:, 4, :],
                            rhs=pab[:, 504 * t + 252 : 504 * t + 756],
                            start=False,
                            stop=True,
                        )
                    # m = tanh(psum + (-bias)): evacuate one 2-tile block
                    nc.scalar.activation(
                        out=m[:, blk * 1008 : (blk + 1) * 1008],
                        in_=pcb[:, :, 0:504],
                        func=mybir.ActivationFunctionType.Tanh,
                        bias=bt[:],
                    )

                # --- GpSimd route: px 0..PSPLIT ---
                r = rpool.tile([128, PSPLIT], F16, tag="r")
                nc.gpsimd.partition_all_reduce(
                    r[:],
                    m[:, 0:PSPLIT],
                    channels=128,
                    reduce_op=bass_isa.ReduceOp.max,
                )
                # scatter row 0 across partitions via a DRAM round-trip
                nc.sync.dma_start(out=scratch.ap(), in_=r[0:1, :])
                s = spool.tile([96, WO], F16, tag="s")
                nc.sync.dma_start(out=s[:], in_=scratch.ap())
                f = fpool.tile([96, WO], F16, tag="f")
                nc.scalar.activation(
                    out=f[:],
                    in_=s[:],
                    func=mybir.ActivationFunctionType.Tanh,
                    scale=-1.0,
                )
                nc.sync.dma_start(out=out.ap()[img, 0:PSPLIT], in_=f[:])

                # --- PE route: px PSPLIT..16128 (valid to 15875) ---
                mall = gpool.tile([128, 32], F16, tag="mall")
                nc.vector.memset(mall[:], 0.0)
                for k in range(32):
                    c0 = PSPLIT + 128 * k
                    wdt = 128 if k < 31 else 64
                    tp = tpool.tile([128, 128], F16, tag="tp")
                    nc.tensor.transpose(
                        out=tp[0:wdt, :], in_=m[:, c0 : c0 + wdt], identity=idt[:]
                    )
                    nc.vector.tensor_reduce(
                        out=mall[0:wdt, k : k + 1],
                        in_=tp[0:wdt, :],
                        axis=mybir.AxisListType.X,
                        op=mybir.AluOpType.max,
                    )
                g = gpool.tile([128, 32], F16, tag="g")
                nc.scalar.activation(
                    out=g[:],
                    in_=mall[:],
                    func=mybir.ActivationFunctionType.Tanh,
                    scale=-1.0,
                )
                gtp = gtpool.tile([32, 128], F16, tag="gtp")
                nc.tensor.transpose(out=gtp[:], in_=g[:, 0:32], identity=idt[:])
                gt = spool.tile([32, 128], F16, tag="gt")
                nc.vector.tensor_copy(out=gt[:], in_=gtp[:])
                # rows are 128-px chunks from PSPLIT: 29 full + 68 px
                nc.sync.dma_start(
                    out=out.ap()[img, PSPLIT : PSPLIT + 29 * 128], in_=gt[0:29, :]
                )
                nc.sync.dma_start(
                    out=out.ap()[img, PSPLIT + 29 * 128 : NPIX], in_=gt[29:30, 0:68]
                )
    nc.compile()
    return nc


def prep_inputs(x, weight, bias):
    """Host-side packing -> per-core input maps (list of 8 dicts)."""
    x = np.asarray(x, dtype=np.float32)
    weight = np.asarray(weight, dtype=np.float32)
    bias = np.asarray(bias, dtype=np.float32)
    f8np = mybir.dt.np(F8)

    # dup row-block layouts [N, 128, NBLK, 126]
    pab = np.zeros((N_IMGS, 128, NBLK, WO), dtype=np.float32)
    pct = np.zeros((N_IMGS, 128, NBLK, WO), dtype=np.float32)
    pab[:, 0:64, 0:H, :] = x[:, :, :, 0:WO]
    pab[:, 64:128, 0:H, :] = x[:, :, :, 1 : 1 + WO]
    pct[:, 0:64, 0:H, :] = x[:, :, :, 2 : 2 + WO]
    pct[:, 64:128, 0 : H - 1, :] = x[:, :, 1:, 2 : 2 + WO]
    pab = pab.reshape(N_IMGS, 128, PITCH).astype(f8np)
    pct = pct.reshape(N_IMGS, 128, PITCH).astype(f8np)

    wneg = -weight
    wp = np.zeros((128, 5, 128), dtype=np.float32)
    # slots 0,1: DoubleRow planes for P_AB -> taps (0,0),(0,1) / (1,0),(1,1)
    # slots 2,3: DoubleRow planes for P_C  -> taps (0,2),(1,2) / -, (2,2)
    # slot 4: plain for P_AB at +252       -> taps (2,0),(2,1)
    wp[0:64, 0] = wneg[:, :, 0, 0].T
    wp[64:128, 0] = wneg[:, :, 0, 1].T
    wp[0:64, 1] = wneg[:, :, 1, 0].T
    wp[64:128, 1] = wneg[:, :, 1, 1].T
    wp[0:64, 2] = wneg[:, :, 0, 2].T
    wp[64:128, 2] = wneg[:, :, 1, 2].T
    wp[64:128, 3] = wneg[:, :, 2, 2].T
    wp[0:64, 4] = wneg[:, :, 2, 0].T
    wp[64:128, 4] = wneg[:, :, 2, 1].T
    wp = wp.astype(f8np)

    b2 = -bias.reshape(128, 1).astype(np.float32)
    idn = np.eye(128, dtype=np.float16)

    in_maps = []
    for c in range(N_CORES):
        sl = slice(c * IMGS_PER_CORE, (c + 1) * IMGS_PER_CORE)
        in_maps.append(
            {
                "pab": np.ascontiguousarray(pab[sl]),
                "pcd": np.ascontiguousarray(pct[sl]),
                "wp": wp,
                "bias": b2,
                "ident": idn,
            }
        )
    return in_maps


def timing_in_maps():
    """Inputs for the timing=True variant: only the tiny replicated consts."""
    f8np = mybir.dt.np(F8)
    return [
        {
            "wp": np.zeros((128, 5, 128), dtype=f8np),
            "bias": np.zeros((128, 1), dtype=np.float32),
            "ident": np.eye(128, dtype=np.float16),
        }
    ] * N_CORES


def assemble_output(results):
    """results: list of 8 per-core out dicts -> full [32, 1, 126, 126] f32."""
    parts = [np.asarray(results[c]["out"], dtype=np.float32) for c in range(N_CORES)]
    full = np.concatenate(parts, axis=0)  # [32, 15876]
    return full.reshape(N_IMGS, 1, HO, WO)


_NC_CACHE = None


def kernel(x, weight, bias):
    global _NC_CACHE
    if _NC_CACHE is None:
        _NC_CACHE = build_kernel()
    in_maps = prep_inputs(x, weight, bias)
    res = run_bass_kernel_spmd(_NC_CACHE, in_maps, list(range(N_CORES)))
    return assemble_output(res.results)


# revision 6
# speedup vs baseline: 2.2580x; 2.2580x over previous
"""Trainium2 Bass kernel v6: conv2d(3x3, VALID) + bias -> channel-min -> tanh(tanh).

Problem shapes (fixed):
  x      [32, 64, 128, 128] f32   (N, C_in, H, W)
  weight [128, 64, 3, 3]    f32   (C_out, C_in, kh, kw)
  bias   [128]              f32
  out    [32, 1, 126, 126]  f32

Strategy (v6: fp8 DoubleRow conv; channel-min split between GpSimd and PE)
--------------------------------------------------------------------------
Data-parallel over 8 cores: 4 images per core, weights/bias replicated.

Conv identical to v5: compact fp8 dup row-block layouts P_AB / P_C
(col = r*126 + w), 3 accumulating matmuls per 504-px tile (2x DoubleRow
K=256 with an overlapping [[pitch,128],[126,2],[1,504]] rhs AP + 1 plain
K=128), negated weights; ScalarE evacuates PSUM per 2-tile block with
m = tanh(-(y+b)) (f32 bias).

Channel-min (min = -max of negated, tanh monotone+odd), split to balance
engines (GpSimd partition_all_reduce has a high fixed+per-element cost;
TensorE has headroom):
  px 0..12095      GpSimdE: one partition_all_reduce(max) per image,
                   scatter row 0 via DRAM round-trip, ScalarE tanh(-.),
                   one output DMA.
  px 12096..15875  TensorE: transpose 128-px chunks of m (channels -> free
                   dim), VectorE max-reduce per chunk, ScalarE tanh(-.),
                   one more transpose so pixels are contiguous, DMA out.
"""

import numpy as np

import concourse.bacc as bacc
import concourse.bass as bass
import concourse.bass_isa as bass_isa
import concourse.tile as tile
from concourse import mybir
from concourse.bass_utils import run_bass_kernel_spmd

N_CORES = 8
N_IMGS = 32
IMGS_PER_CORE = N_IMGS // N_CORES
C_IN = 64
C_OUT = 128
H = W = 128
HO = WO = 126
NPIX = HO * WO  # 15876
NBLK = 131  # row blocks in the dup layouts (126..130 only partially used)
PITCH = NBLK * WO  # 16506
PSPLIT = 10080  # px 0..PSPLIT-1 via GpSimd (80 rows), rest via PE transposes
F8 = mybir.dt.float8e4
F16 = mybir.dt.float16
F32 = mybir.dt.float32


def build_kernel(reps=1, timing=False):
    """reps > 1 repeats the whole per-core compute in one NEFF (for HW timing).

    timing=True declares the big image inputs and the result as Internal DRAM
    (zero-initialized on device at NEFF start) so per-call host<->device
    transfer is tiny; the per-rep instruction stream is identical."""
    nc = bacc.Bacc(trn_type="TRN2", target_bir_lowering=False, debug=False)
    io_kind = "Internal" if timing else None
    pab_d = nc.dram_tensor(
        "pab", [IMGS_PER_CORE, 128, PITCH], F8, kind=io_kind or "ExternalInput"
    )
    pc_d = nc.dram_tensor(
        "pcd", [IMGS_PER_CORE, 128, PITCH], F8, kind=io_kind or "ExternalInput"
    )
    wp = nc.dram_tensor("wp", [128, 5, 128], F8, kind="ExternalInput")
    bias = nc.dram_tensor("bias", [128, 1], F32, kind="ExternalInput")
    ident = nc.dram_tensor("ident", [128, 128], F16, kind="ExternalInput")
    out = nc.dram_tensor(
        "out", [IMGS_PER_CORE, NPIX], F16, kind=io_kind or "ExternalOutput"
    )
    sink = (
        nc.dram_tensor("sink", [1, 64], F32, kind="ExternalOutput") if timing else None
    )
    scratch = nc.dram_tensor("scratch", [80, 126], F16, kind="Internal")

    DR = mybir.MatmulPerfMode.DoubleRow

    with tile.TileContext(nc) as tc:
        with (
            tc.tile_pool(name="consts", bufs=1) as consts,
            tc.tile_pool(name="dpool", bufs=2) as dpool,
            tc.tile_pool(name="mpool", bufs=2) as mpool,
            tc.tile_pool(name="rpool", bufs=2) as rpool,
            tc.tile_pool(name="spool", bufs=2) as spool,
            tc.tile_pool(name="fpool", bufs=2) as fpool,
            tc.tile_pool(name="gpool", bufs=2) as gpool,
            tc.tile_pool(name="pcpool", bufs=2, space="PSUM") as pcpool,
            tc.tile_pool(name="tpool", bufs=2, space="PSUM") as tpool,
            tc.tile_pool(name="gtpool", bufs=2, space="PSUM") as gtpool,
        ):
            wpt = consts.tile([128, 5, 128], F8)
            nc.gpsimd.dma_start(out=wpt[:], in_=wp.ap())
            bt = consts.tile([128, 1], F32)
            nc.gpsimd.dma_start(out=bt[:], in_=bias.ap())
            idt = consts.tile([128, 128], F16)
            nc.gpsimd.dma_start(out=idt[:], in_=ident.ap())

            if timing:
                # one-time (outside the rep loop): zero-fill the internal
                # image inputs; produce the tiny external output
                z = dpool.tile([128, PITCH], F8, tag="pab")
                for q in range(2):
                    nc.vector.memset(z[:, q * 8253 : (q + 1) * 8253], 0.0)
                for img in range(IMGS_PER_CORE):
                    nc.sync.dma_start(out=pab_d.ap()[img], in_=z[:])
                    nc.sync.dma_start(out=pc_d.ap()[img], in_=z[:])
                zs = fpool.tile([1, 64], F32, tag="sink")
                nc.vector.memset(zs[:], 0.0)
                nc.sync.dma_start(out=sink.ap(), in_=zs[:])

            for img in [i for _ in range(reps) for i in range(IMGS_PER_CORE)]:
                pab = dpool.tile([128, PITCH], F8, tag="pab")
                nc.sync.dma_start(out=pab[:], in_=pab_d.ap()[img])
                pct = dpool.tile([128, PITCH], F8, tag="pct")
                nc.sync.dma_start(out=pct[:], in_=pc_d.ap()[img])
                pab_t = pab[:, 0:504].tensor
                pct_t = pct[:, 0:504].tensor

                m = mpool.tile([128, 32 * 504], F16, tag="m")
                for blk in range(16):
                    pcb = pcpool.tile([128, 2, 512], F32, tag="pc")
                    for j in range(2):
                        t = blk * 2 + j
                        rhs_a = bass.AP(
                            tensor=pab_t,
                            offset=504 * t,
                            ap=[[PITCH, 128], [WO, 2], [1, 504]],
                        )
                        rhs_b = bass.AP(
                            tensor=pct_t,
                            offset=504 * t,
                            ap=[[PITCH, 128], [WO, 2], [1, 504]],
                        )
                        nc.tensor.matmul(
                            pcb[:, j, 0:504],
                            lhsT=wpt[:, 0:2, :],
                            rhs=rhs_a,
                            start=True,
                            stop=False,
                            perf_mode=DR,
                        )
                        nc.tensor.matmul(
                            pcb[:, j, 0:504],
                            lhsT=wpt[:, 2:4, :],
                            rhs=rhs_b,
                            start=False,
                            stop=False,
                            perf_mode=DR,
                        )
                        nc.tensor.matmul(
                            pcb[:, j, 0:504],
                            lhsT=wpt[:, 4, :],
                            rhs=pab[:, 504 * t + 252 : 504 * t + 756],
                            start=False,
                            stop=True,
                        )
                    # m = tanh(psum + (-bias)): evacuate one 2-tile block
                    nc.scalar.activation(
                        out=m[:, blk * 1008 : (blk + 1) * 1008],
                        in_=pcb[:, :, 0:504],
                        func=mybir.ActivationFunctionType.Tanh,
                        bias=bt[:],
                    )

                # --- GpSimd route: px 0..PSPLIT ---
                r = rpool.tile([128, PSPLIT], F16, tag="r")
                nc.gpsimd.partition_all_reduce(
                    r[:],
                    m[:, 0:PSPLIT],
                    channels=128,
                    reduce_op=bass_isa.ReduceOp.max,
                )
                # scatter row 0 across partitions via a DRAM round-trip
                nc.sync.dma_start(out=scratch.ap(), in_=r[0:1, :])
                s = spool.tile([80, WO], F16, tag="s")
                nc.sync.dma_start(out=s[:], in_=scratch.ap())
                f = fpool.tile([80, WO], F16, tag="f")
                nc.scalar.activation(
                    out=f[:],
                    in_=s[:],
                    func=mybir.ActivationFunctionType.Tanh,
                    scale=-1.0,
                )
                nc.sync.dma_start(out=out.ap()[img, 0:PSPLIT], in_=f[:])

                # --- PE route: px PSPLIT..16128 (valid to 15875) ---
                mall = gpool.tile([128, 48], F16, tag="mall")
                nc.vector.memset(mall[:], 0.0)
                for k in range(48):
                    c0 = PSPLIT + 128 * k
                    wdt = 128 if k < 47 else 32
                    tp = tpool.tile([128, 128], F16, tag="tp")
                    nc.tensor.transpose(
                        out=tp[0:wdt, :], in_=m[:, c0 : c0 + wdt], identity=idt[:]
                    )
                    nc.vector.tensor_reduce(
                        out=mall[0:wdt, k : k + 1],
                        in_=tp[0:wdt, :],
                        axis=mybir.AxisListType.X,
                        op=mybir.AluOpType.max,
                    )
                g = gpool.tile([128, 48], F16, tag="g")
                nc.scalar.activation(
                    out=g[:],
                    in_=mall[:],
                    func=mybir.ActivationFunctionType.Tanh,
                    scale=-1.0,
                )
                gtp = gtpool.tile([48, 128], F16, tag="gtp")
                nc.tensor.transpose(out=gtp[:], in_=g[:, 0:48], identity=idt[:])
                gt = spool.tile([48, 128], F16, tag="gt")
                nc.vector.tensor_copy(out=gt[:], in_=gtp[:])
                # rows are 128-px chunks from PSPLIT: 45 full + 36 px
                nc.sync.dma_start(
                    out=out.ap()[img, PSPLIT : PSPLIT + 45 * 128], in_=gt[0:45, :]
                )
                nc.sync.dma_start(
                    out=out.ap()[img, PSPLIT + 45 * 128 : NPIX], in_=gt[45:46, 0:36]
                )
    nc.compile()
    return nc


def prep_inputs(x, weight, bias):
    """Host-side packing -> per-core input maps (list of 8 dicts)."""
    x = np.asarray(x, dtype=np.float32)
    weight = np.asarray(weight, dtype=np.float32)
    bias = np.asarray(bias, dtype=np.float32)
    f8np = mybir.dt.np(F8)

    # dup row-block layouts [N, 128, NBLK, 126]
    pab = np.zeros((N_IMGS, 128, NBLK, WO), dtype=np.float32)
    pct = np.zeros((N_IMGS, 128, NBLK, WO), dtype=np.float32)
    pab[:, 0:64, 0:H, :] = x[:, :, :, 0:WO]
    pab[:, 64:128, 0:H, :] = x[:, :, :, 1 : 1 + WO]
    pct[:, 0:64, 0:H, :] = x[:, :, :, 2 : 2 + WO]
    pct[:, 64:128, 0 : H - 1, :] = x[:, :, 1:, 2 : 2 + WO]
    pab = pab.reshape(N_IMGS, 128, PITCH).astype(f8np)
    pct = pct.reshape(N_IMGS, 128, PITCH).astype(f8np)

    wneg = -weight
    wp = np.zeros((128, 5, 128), dtype=np.float32)
    # slots 0,1: DoubleRow planes for P_AB -> taps (0,0),(0,1) / (1,0),(1,1)
    # slots 2,3: DoubleRow planes for P_C  -> taps (0,2),(1,2) / -, (2,2)
    # slot 4: plain for P_AB at +252       -> taps (2,0),(2,1)
    wp[0:64, 0] = wneg[:, :, 0, 0].T
    wp[64:128, 0] = wneg[:, :, 0, 1].T
    wp[0:64, 1] = wneg[:, :, 1, 0].T
    wp[64:128, 1] = wneg[:, :, 1, 1].T
    wp[0:64, 2] = wneg[:, :, 0, 2].T
    wp[64:128, 2] = wneg[:, :, 1, 2].T
    wp[64:128, 3] = wneg[:, :, 2, 2].T
    wp[0:64, 4] = wneg[:, :, 2, 0].T
    wp[64:128, 4] = wneg[:, :, 2, 1].T
    wp = wp.astype(f8np)

    b2 = -bias.reshape(128, 1).astype(np.float32)
    idn = np.eye(128, dtype=np.float16)

    in_maps = []
    for c in range(N_CORES):
        sl = slice(c * IMGS_PER_CORE, (c + 1) * IMGS_PER_CORE)
        in_maps.append(
            {
                "pab": np.ascontiguousarray(pab[sl]),
                "pcd": np.ascontiguousarray(pct[sl]),
                "wp": wp,
                "bias": b2,
                "ident": idn,
            }
        )
    return in_maps


def timing_in_maps():
    """Inputs for the timing=True variant: only the tiny replicated consts."""
    f8np = mybir.dt.np(F8)
    return [
        {
            "wp": np.zeros((128, 5, 128), dtype=f8np),
            "bias": np.zeros((128, 1), dtype=np.float32),
            "ident": np.eye(128, dtype=np.float16),
        }
    ] * N_CORES


def assemble_output(results):
    """results: list of 8 per-core out dicts -> full [32, 1, 126, 126] f32."""
    parts = [np.asarray(results[c]["out"], dtype=np.float32) for c in range(N_CORES)]
    full = np.concatenate(parts, axis=0)  # [32, 15876]
    return full.reshape(N_IMGS, 1, HO, WO)


_NC_CACHE = None


def kernel(x, weight, bias):
    global _NC_CACHE
    if _NC_CACHE is None:
        _NC_CACHE = build_kernel()
    in_maps = prep_inputs(x, weight, bias)
    res = run_bass_kernel_spmd(_NC_CACHE, in_maps, list(range(N_CORES)))
    return assemble_output(res.results)
